# revision 28
# baseline (speedup 1.0000x reference)
"""Sigmoid-attention block kernel for trn2 (one NeuronCore, SPMD over 8).

fp8-DoubleRow attention with decoupled PSUM rings (~110us HW, rel err
1.3e-2 vs the 2e-2 gate). Host prep: queryT [H, SLAB] bf16, valueT
[H, N] bf16, WqT/WvT [H, H] bf16 (= W.T), keyT8 [128, 2, N] e4m3 and
WkT8 [128, 2, H] e4m3 (DoubleRow interleave: plane o = contraction rows
128o..128o+127), bq/bk [H] fp32, bv2 [128, 512] fp32 (bv twice).

Per core:
  qT   = Wq-blocks @ queryT + bq      (bf16)     -> e4m3 [128, 2, SLAB]
  kT   = Wk .T2 @ keyT8 + bk          (fp8 DR)   -> e4m3 [128, 2, N]
  vp   = valueT-blocks @ WvT + bv     (bf16)     -> e4m3 [128, 2, H]/pair
  attnT[j] = sigmoid(kT .T2 @ qT)     (fp8 DR; ACT sigmoid [128,1024]
             psum -> e4m3; all 32 pair-tiles retained in SBUF)
  outT[0:128]   += vp .T2 @ attnT     (fp8 DR, inline over 32 pairs)
  outT[128:256] += vp .T2 @ attnT     (fp8 DR, phase B re-reading the
             retained attnT tiles)

.T2 marks perf_mode=DoubleRow matmuls: both operands [128, 2, cols]
(contraction 256 in one instruction, 2 fp8/cycle/lane moving stream).

PSUM rings (8 banks): psL 2x[128,1024] carries ONLY logits psums, so
each logits grab waits exactly one sigmoid two grabs back — the ACT
sigmoid stream (the ~71us floor engine) is the pacemaker with no slow
DVE consumer ever in its ring. psP 2x[128,512] carries the kT/vproj
projection psums (fast DVE/ACT consumers), and in phase B its two slots
become the h-block-1 output accumulators; the Tile scheduler hoists
most phase-B matmuls into the last two chunks' PE slack. psO
1x[128,1024] is the inline h-block-0 accumulator (its slot also hosts
the qT projection psums during the prologue).
"""
from contextlib import ExitStack

import concourse.bass as bass
import concourse.mybir as mybir
import concourse.tile as tile
from concourse import bacc
from concourse.masks import make_identity

F32 = mybir.dt.float32
BF16 = mybir.dt.bfloat16
E4 = mybir.dt.float8e4
AF = mybir.ActivationFunctionType
DR = mybir.MatmulPerfMode.DoubleRow


def _build_attn_kernel(SLAB=1024, N=8192, H=256):
    assert H == 256
    HB = H // 128            # 2 h-blocks
    NJ = N // 128            # 64 j-blocks
    NP = NJ // 2             # 32 j-pairs
    CW = 1024                # key/value chunk width (j cols)
    NCH = N // CW            # 8 chunks
    PPC = CW // 256          # 4 pairs per chunk
    IC = SLAB // 512         # 2 i sub-blocks

    nc = bacc.Bacc()
    queryT = nc.dram_tensor("queryT", [H, SLAB], BF16, kind="ExternalInput")
    keyT8 = nc.dram_tensor("keyT8", [128, 2, N], E4, kind="ExternalInput")
    valueT = nc.dram_tensor("valueT", [H, N], BF16, kind="ExternalInput")
    WqT = nc.dram_tensor("WqT", [H, H], BF16, kind="ExternalInput")
    WkT8 = nc.dram_tensor("WkT8", [128, 2, H], E4, kind="ExternalInput")
    WvT = nc.dram_tensor("WvT", [H, H], BF16, kind="ExternalInput")
    bq = nc.dram_tensor("bq", [H], F32, kind="ExternalInput")
    bk = nc.dram_tensor("bk", [H], F32, kind="ExternalInput")
    bv2 = nc.dram_tensor("bv2", [128, 512], F32, kind="ExternalInput")
    outd = nc.dram_tensor("outT", [H, SLAB], F32, kind="ExternalOutput")

    with tile.TileContext(nc) as tc, ExitStack() as ctx:
        cpool = ctx.enter_context(tc.tile_pool(name="const", bufs=1))
        psO = ctx.enter_context(tc.tile_pool(name="psO", bufs=1, space="PSUM"))
        psL = ctx.enter_context(tc.tile_pool(name="psL", bufs=2, space="PSUM"))
        psP = ctx.enter_context(tc.tile_pool(name="psP", bufs=2, space="PSUM"))
        kqp = ctx.enter_context(tc.tile_pool(name="kqp", bufs=1))
        atp = ctx.enter_context(tc.tile_pool(name="atp", bufs=NP + 2))
        vpp = ctx.enter_context(tc.tile_pool(name="vpp", bufs=NP + 2))
        kcp = ctx.enter_context(tc.tile_pool(name="kcp", bufs=3))
        vcp = ctx.enter_context(tc.tile_pool(name="vcp", bufs=3))
        outp = ctx.enter_context(tc.tile_pool(name="outp", bufs=1))

        ident = cpool.tile([128, 128], F32, tag="ident")
        make_identity(nc, ident[:])
        # sigmoid table preload, first thing on the ACT queue
        sgw = cpool.tile([128, 1], F32, tag="sgw")
        nc.scalar.activation(sgw[:], ident[:, :1], AF.Sigmoid)

        # ---- head DMAs: ALL on sync, ordered by first use ----
        qf, wq_sb, wv_sb = [], [], []
        for hpb in range(HB):
            q = cpool.tile([128, SLAB], BF16, tag=f"qf{hpb}", name=f"qf{hpb}")
            nc.sync.dma_start(q[:], queryT[hpb * 128:(hpb + 1) * 128, :])
            qf.append(q)
        for hpb in range(HB):
            wq = cpool.tile([128, H], BF16, tag=f"wq{hpb}", name=f"wq{hpb}")
            nc.sync.dma_start(wq[:], WqT[hpb * 128:(hpb + 1) * 128, :])
            wq_sb.append(wq)
        bq_t, bk_t = [], []
        for hb in range(HB):
            b = cpool.tile([128, 1], F32, tag=f"bq{hb}", name=f"bq{hb}")
            nc.sync.dma_start(b[:], bq[hb * 128:(hb + 1) * 128][:, None])
            bq_t.append(b)
        # chunk-0 key/value loads go ahead of the remaining constants
        kc0 = kcp.tile([128, 2, CW], E4, tag="kc", name="kc0")
        nc.sync.dma_start(kc0[:], keyT8[:, :, 0:CW])
        wk8 = cpool.tile([128, 2, H], E4, tag="wk8")
        nc.sync.dma_start(wk8[:], WkT8[:, :, :])
        for hb in range(HB):
            b2 = cpool.tile([128, 1], F32, tag=f"bk{hb}", name=f"bk{hb}")
            nc.sync.dma_start(b2[:], bk[hb * 128:(hb + 1) * 128][:, None])
            bk_t.append(b2)
        vc0 = []
        for hpb in range(HB):
            v = vcp.tile([128, CW], BF16, tag=f"vc{hpb}", name=f"vc{hpb}")
            nc.sync.dma_start(v[:], valueT[hpb * 128:(hpb + 1) * 128, 0:CW])
            vc0.append(v)
        for hpb in range(HB):
            wv = cpool.tile([128, H], BF16, tag=f"wv{hpb}", name=f"wv{hpb}")
            nc.sync.dma_start(wv[:], WvT[hpb * 128:(hpb + 1) * 128, :])
            wv_sb.append(wv)
        bv2_t = cpool.tile([128, 512], F32, tag="bv2")
        nc.sync.dma_start(bv2_t[:], bv2[:, :])

        # short HAM warmup into a psL slot; the prologue projections
        # continue the PE stream without a gap
        pwarm = psL.tile([128, 1024], F32, tag="ps", name="pwarm")
        for r in range(6):
            nc.tensor.matmul(pwarm[:, :128], ident[:], ident[:],
                             start=True, stop=True)

        # ---- qT projection -> e4m3, in the (still free) psO slot;
        # bias-add + fp8 casts on ACT (same table set as sigmoid)
        qT_f8 = kqp.tile([128, 2, SLAB], E4, tag="qT_f8")
        for hb in range(HB):
            pq = psO.tile([128, SLAB], F32, tag="po", name=f"pq{hb}")
            for ic in range(IC):
                for hpb in range(HB):
                    nc.tensor.matmul(
                        pq[:, ic * 512:(ic + 1) * 512],
                        wq_sb[hpb][:, hb * 128:(hb + 1) * 128],
                        qf[hpb][:, ic * 512:(ic + 1) * 512],
                        start=(hpb == 0), stop=(hpb == HB - 1),
                    )
            if hb == 0:
                nc.scalar.add(qT_f8[:, hb, :], pq[:, :], bq_t[hb][:])
            else:
                nc.vector.tensor_scalar_add(
                    qT_f8[:, hb, :], pq[:, :], bq_t[hb][:]
                )

        kT_f8 = kqp.tile([128, 2, N], E4, tag="kT_f8")
        vp_tiles = [None] * NP
        at_tiles = [None] * NP

        def emit_kv_dma(c):
            kc = kcp.tile([128, 2, CW], E4, tag="kc", name="kc")
            nc.sync.dma_start(kc[:], keyT8[:, :, c * CW:(c + 1) * CW])
            vc = []
            for hpb in range(HB):
                v = vcp.tile([128, CW], BF16, tag=f"vc{hpb}", name=f"vc{hpb}")
                nc.sync.dma_start(
                    v[:], valueT[hpb * 128:(hpb + 1) * 128, c * CW:(c + 1) * CW]
                )
                vc.append(v)
            return kc, vc

        def emit_kT_quarter(c, kc, q, on_act=False, pool=None):
            """Project (h-block, s-half) q of key chunk c: one psum grab."""
            hb, s = q // 2, q % 2
            if pool is None:
                pk = psP.tile([128, 512], F32, tag="pp", name="pk")
            else:
                pk = pool.tile([128, 1024], F32, tag="ps", name="pkL")[:, 0:512]
            nc.tensor.matmul(
                pk[:, :],
                wk8[:, :, hb * 128:(hb + 1) * 128],
                kc[:, :, s * 512:(s + 1) * 512],
                start=True, stop=True, perf_mode=DR,
            )
            lo = c * CW + s * 512
            if on_act:
                nc.scalar.add(kT_f8[:, hb, lo:lo + 512], pk[:, :], bk_t[hb][:])
            else:
                nc.vector.tensor_scalar_add(
                    kT_f8[:, hb, lo:lo + 512], pk[:, :], bk_t[hb][:]
                )

        def emit_v_pair(c, vc, t, pool=None):
            """Project j-pair t of value chunk c: one psum grab."""
            if pool is None:
                pv = psP.tile([128, 512], F32, tag="pp", name="pv")
            else:
                pv = pool.tile([128, 1024], F32, tag="ps", name="pvL")[:, 0:512]
            for o in range(2):
                jl = 2 * t + o
                for hpb in range(HB):
                    nc.tensor.matmul(
                        pv[:, o * 256:(o + 1) * 256],
                        vc[hpb][:, jl * 128:(jl + 1) * 128],
                        wv_sb[hpb][:, :],
                        start=(hpb == 0), stop=(hpb == HB - 1),
                    )
            p = c * PPC + t
            vp = vpp.tile([128, 2, H], E4, tag="vp", name="vp")
            nc.vector.tensor_add(vp[:, :, :], pv[:, :], bv2_t[:, :])
            vp_tiles[p] = vp

        def emit_pair_logits(p):
            at = atp.tile([128, 2, SLAB], E4, tag="at", name="at")
            for o in range(2):
                j = 2 * p + o
                pl = psL.tile([128, 1024], F32, tag="ps", name="pl")
                for ic in range(IC):
                    nc.tensor.matmul(
                        pl[:, ic * 512:(ic + 1) * 512],
                        kT_f8[:, :, j * 128:(j + 1) * 128],
                        qT_f8[:, :, ic * 512:(ic + 1) * 512],
                        start=True, stop=True, perf_mode=DR,
                    )
                nc.scalar.activation(at[:, o, :], pl[:, :], AF.Sigmoid)
            at_tiles[p] = at

        def emit_out_acc_hb0(p):
            at, vp = at_tiles[p], vp_tiles[p]
            for ic in range(IC):
                nc.tensor.matmul(
                    po[:, ic * 512:(ic + 1) * 512],
                    vp[:, :, 0:128],
                    at[:, :, ic * 512:(ic + 1) * 512],
                    start=(p == 0), stop=(p == NP - 1), perf_mode=DR,
                )

        # ---- prologue: chunk 0 projections (DMAs already in flight);
        # casts split across ACT and DVE to shorten the serial chain ----
        # s0 quarters (both h-blocks) and the first v-pairs come first:
        # they are all pair-0/1's logits need, so the first sigmoid fires
        # after two casts instead of four. k0 on ACT, the rest on DVE.
        emit_kT_quarter(0, kc0, 0, on_act=True)
        emit_kT_quarter(0, kc0, 2)
        emit_v_pair(0, vc0, 0)
        emit_v_pair(0, vc0, 1)
        emit_kT_quarter(0, kc0, 1)
        emit_kT_quarter(0, kc0, 3)
        emit_v_pair(0, vc0, 2)
        emit_v_pair(0, vc0, 3)

        # the inline h-block-0 accumulator reuses the qT psum slot
        po = psO.tile([128, SLAB], F32, tag="po", name="po")

        # ---- main loop: psL carries only logits (sigmoid-paced ring);
        # one kT-quarter and one v-pair of chunk c+1 slot in per pair ----
        for c in range(NCH):
            nxt = None
            if c + 1 < NCH:
                nxt = emit_kv_dma(c + 1)
            for t in range(PPC):
                p = c * PPC + t
                emit_pair_logits(p)
                if p >= 1:
                    emit_out_acc_hb0(p - 1)
                if nxt is not None:
                    kc, vc = nxt
                    emit_kT_quarter(c + 1, kc, t)
                    emit_v_pair(c + 1, vc, t)
        emit_out_acc_hb0(NP - 1)

        # ---- phase B: h-block-1 accumulation in the freed psP slots,
        # re-reading the retained attnT/vp tiles ----
        po1 = [psP.tile([128, 512], F32, tag="pp", name=f"po1_{ic}")
               for ic in range(IC)]
        for p in range(NP):
            at, vp = at_tiles[p], vp_tiles[p]
            for ic in range(IC):
                nc.tensor.matmul(
                    po1[ic][:, :],
                    vp[:, :, 128:256],
                    at[:, :, ic * 512:(ic + 1) * 512],
                    start=(p == 0), stop=(p == NP - 1), perf_mode=DR,
                )

        # ---- drain: half-copies split DVE/ACT, 4 DMAs on 2 queues ----
        ot = [outp.tile([128, SLAB], F32, tag=f"ot{hb}", name=f"ot{hb}")
              for hb in range(HB)]
        nc.vector.tensor_copy(ot[0][:, 0:512], po[:, 0:512])
        nc.scalar.copy(ot[0][:, 512:1024], po[:, 512:1024])
        nc.vector.tensor_copy(ot[1][:, 0:512], po1[0][:, :])
        nc.scalar.copy(ot[1][:, 512:1024], po1[1][:, :])
        for hb in range(HB):
            dma = nc.sync if hb == 0 else nc.scalar
            for s in range(2):
                dma.dma_start(
                    outd[hb * 128:(hb + 1) * 128, s * 512:(s + 1) * 512],
                    ot[hb][:, s * 512:(s + 1) * 512],
                )

    nc.finalize()
    return nc


import numpy as np
import ml_dtypes
from concourse.bass_utils import run_bass_kernel_spmd

BF = ml_dtypes.bfloat16
N_CORES = 8
N_FULL = 8192
H_FULL = 256
SLAB_FULL = N_FULL // N_CORES

_NC = None


def _get_nc():
    global _NC
    if _NC is None:
        _NC = _build_attn_kernel(SLAB=SLAB_FULL, N=N_FULL, H=H_FULL)
    return _NC


def _in_maps(inputs):
    import concourse.mybir as mybir
    E4NP = mybir.dt.np(mybir.dt.float8e4)
    full = {k: np.asarray(v, dtype=np.float32) for k, v in inputs.items()}
    queryT = np.ascontiguousarray(full["query"].T.astype(BF))  # [H, N]
    N, H = full["key"].shape[0], full["key"].shape[1]
    # key/Wk in fp8 DoubleRow layout [128, 2, cols]: plane o holds
    # contraction rows 128o..128o+127
    keyT8 = full["key"].T.reshape(2, 128, N).transpose(1, 0, 2)
    WkT8 = full["Wk"].T.reshape(2, 128, H).transpose(1, 0, 2)
    shared = {
        "keyT8": np.ascontiguousarray(keyT8.astype(E4NP)),
        "valueT": np.ascontiguousarray(full["value"].T.astype(BF)),
        "WqT": np.ascontiguousarray(full["Wq"].T.astype(BF)),
        "WkT8": np.ascontiguousarray(WkT8.astype(E4NP)),
        "WvT": np.ascontiguousarray(full["Wv"].T.astype(BF)),
        "bq": np.ascontiguousarray(full["bq"]),
        "bk": np.ascontiguousarray(full["bk"]),
        "bv2": np.ascontiguousarray(np.tile(full["bv"][None, :], (128, 2))),
    }
    maps = []
    for c in range(N_CORES):
        m = dict(shared)
        m["queryT"] = np.ascontiguousarray(
            queryT[:, c * SLAB_FULL:(c + 1) * SLAB_FULL]
        )
        maps.append(m)
    return maps


def kernel(**inputs) -> np.ndarray:
    nc = _get_nc()
    res = run_bass_kernel_spmd(nc, _in_maps(inputs), list(range(N_CORES)))
    return np.ascontiguousarray(np.concatenate(
        [np.asarray(res.results[c]["outT"]).T for c in range(N_CORES)],
        axis=0,
    )).astype(np.float32)


# revision 29
# speedup vs baseline: 1.0113x; 1.0113x over previous
"""Sigmoid-attention block kernel for trn2 (one NeuronCore, SPMD over 8).

fp8-DoubleRow attention with decoupled PSUM rings (~110us HW, rel err
1.3e-2 vs the 2e-2 gate). Host prep: queryT [H, SLAB] bf16, valueT
[H, N] bf16, WqT/WvT [H, H] bf16 (= W.T), keyT8 [128, 2, N] e4m3 and
WkT8 [128, 2, H] e4m3 (DoubleRow interleave: plane o = contraction rows
128o..128o+127), bq/bk [H] fp32, bv2 [128, 512] fp32 (bv twice).

Per core:
  qT   = Wq-blocks @ queryT + bq      (bf16)     -> e4m3 [128, 2, SLAB]
  kT   = Wk .T2 @ keyT8 + bk          (fp8 DR)   -> e4m3 [128, 2, N]
  vp   = valueT-blocks @ WvT + bv     (bf16)     -> e4m3 [128, 2, H]/pair
  attnT[j] = sigmoid(kT .T2 @ qT)     (fp8 DR; ACT sigmoid [128,1024]
             psum -> e4m3; all 32 pair-tiles retained in SBUF)
  outT[0:128]   += vp .T2 @ attnT     (fp8 DR, inline over 32 pairs)
  outT[128:256] += vp .T2 @ attnT     (fp8 DR, phase B re-reading the
             retained attnT tiles)

.T2 marks perf_mode=DoubleRow matmuls: both operands [128, 2, cols]
(contraction 256 in one instruction, 2 fp8/cycle/lane moving stream).

PSUM rings (8 banks): psL 2x[128,1024] carries ONLY logits psums, so
each logits grab waits exactly one sigmoid two grabs back — the ACT
sigmoid stream (the ~71us floor engine) is the pacemaker with no slow
DVE consumer ever in its ring. psP 2x[128,512] carries the kT/vproj
projection psums (fast DVE/ACT consumers), and in phase B its two slots
become the h-block-1 output accumulators; the Tile scheduler hoists
most phase-B matmuls into the last two chunks' PE slack. psO
1x[128,1024] is the inline h-block-0 accumulator (its slot also hosts
the qT projection psums during the prologue).
"""
from contextlib import ExitStack

import concourse.bass as bass
import concourse.mybir as mybir
import concourse.tile as tile
from concourse import bacc
from concourse.masks import make_identity

F32 = mybir.dt.float32
BF16 = mybir.dt.bfloat16
E4 = mybir.dt.float8e4
AF = mybir.ActivationFunctionType
DR = mybir.MatmulPerfMode.DoubleRow


def _build_attn_kernel(SLAB=1024, N=8192, H=256):
    assert H == 256
    HB = H // 128            # 2 h-blocks
    NJ = N // 128            # 64 j-blocks
    NP = NJ // 2             # 32 j-pairs
    CW = 1024                # key/value chunk width (j cols)
    NCH = N // CW            # 8 chunks
    PPC = CW // 256          # 4 pairs per chunk
    IC = SLAB // 512         # 2 i sub-blocks

    nc = bacc.Bacc()
    queryT = nc.dram_tensor("queryT", [H, SLAB], BF16, kind="ExternalInput")
    keyT8 = nc.dram_tensor("keyT8", [128, 2, N], E4, kind="ExternalInput")
    valueT = nc.dram_tensor("valueT", [H, N], BF16, kind="ExternalInput")
    WqT = nc.dram_tensor("WqT", [H, H], BF16, kind="ExternalInput")
    WkT8 = nc.dram_tensor("WkT8", [128, 2, H], E4, kind="ExternalInput")
    WvT = nc.dram_tensor("WvT", [H, H], BF16, kind="ExternalInput")
    bq = nc.dram_tensor("bq", [H], F32, kind="ExternalInput")
    bk = nc.dram_tensor("bk", [H], F32, kind="ExternalInput")
    bv2 = nc.dram_tensor("bv2", [128, 512], F32, kind="ExternalInput")
    outd = nc.dram_tensor("outT", [H, SLAB], F32, kind="ExternalOutput")

    with tile.TileContext(nc) as tc, ExitStack() as ctx:
        cpool = ctx.enter_context(tc.tile_pool(name="const", bufs=1))
        psO = ctx.enter_context(tc.tile_pool(name="psO", bufs=1, space="PSUM"))
        psL = ctx.enter_context(tc.tile_pool(name="psL", bufs=2, space="PSUM"))
        psP = ctx.enter_context(tc.tile_pool(name="psP", bufs=2, space="PSUM"))
        kqp = ctx.enter_context(tc.tile_pool(name="kqp", bufs=1))
        atp = ctx.enter_context(tc.tile_pool(name="atp", bufs=NP + 2))
        vpp = ctx.enter_context(tc.tile_pool(name="vpp", bufs=NP + 2))
        kcp = ctx.enter_context(tc.tile_pool(name="kcp", bufs=3))
        vcp = ctx.enter_context(tc.tile_pool(name="vcp", bufs=3))
        outp = ctx.enter_context(tc.tile_pool(name="outp", bufs=1))

        ident = cpool.tile([128, 128], F32, tag="ident")
        make_identity(nc, ident[:])
        # sigmoid table preload, first thing on the ACT queue
        sgw = cpool.tile([128, 1], F32, tag="sgw")
        nc.scalar.activation(sgw[:], ident[:, :1], AF.Sigmoid)

        # ---- head DMAs: ALL on sync, ordered by first use ----
        qf, wq_sb, wv_sb = [], [], []
        for hpb in range(HB):
            q = cpool.tile([128, SLAB], BF16, tag=f"qf{hpb}", name=f"qf{hpb}")
            nc.sync.dma_start(q[:], queryT[hpb * 128:(hpb + 1) * 128, :])
            qf.append(q)
        for hpb in range(HB):
            wq = cpool.tile([128, H], BF16, tag=f"wq{hpb}", name=f"wq{hpb}")
            nc.sync.dma_start(wq[:], WqT[hpb * 128:(hpb + 1) * 128, :])
            wq_sb.append(wq)
        bq_t, bk_t = [], []
        for hb in range(HB):
            b = cpool.tile([128, 1], F32, tag=f"bq{hb}", name=f"bq{hb}")
            nc.sync.dma_start(b[:], bq[hb * 128:(hb + 1) * 128][:, None])
            bq_t.append(b)
        # chunk-0 key/value loads go ahead of the remaining constants
        kc0 = kcp.tile([128, 2, CW], E4, tag="kc", name="kc0")
        nc.sync.dma_start(kc0[:], keyT8[:, :, 0:CW])
        wk8 = cpool.tile([128, 2, H], E4, tag="wk8")
        nc.sync.dma_start(wk8[:], WkT8[:, :, :])
        for hb in range(HB):
            b2 = cpool.tile([128, 1], F32, tag=f"bk{hb}", name=f"bk{hb}")
            nc.sync.dma_start(b2[:], bk[hb * 128:(hb + 1) * 128][:, None])
            bk_t.append(b2)
        vc0 = []
        for hpb in range(HB):
            v = vcp.tile([128, CW], BF16, tag=f"vc{hpb}", name=f"vc{hpb}")
            nc.sync.dma_start(v[:], valueT[hpb * 128:(hpb + 1) * 128, 0:CW])
            vc0.append(v)
        for hpb in range(HB):
            wv = cpool.tile([128, H], BF16, tag=f"wv{hpb}", name=f"wv{hpb}")
            nc.sync.dma_start(wv[:], WvT[hpb * 128:(hpb + 1) * 128, :])
            wv_sb.append(wv)
        bv2_t = cpool.tile([128, 512], F32, tag="bv2")
        nc.sync.dma_start(bv2_t[:], bv2[:, :])

        # short HAM warmup into a psL slot; the prologue projections
        # continue the PE stream without a gap
        pwarm = psL.tile([128, 1024], F32, tag="ps", name="pwarm")
        for r in range(6):
            nc.tensor.matmul(pwarm[:, :128], ident[:], ident[:],
                             start=True, stop=True)

        # ---- qT projection -> e4m3, in the (still free) psO slot;
        # bias-add + fp8 casts on ACT (same table set as sigmoid)
        qT_f8 = kqp.tile([128, 2, SLAB], E4, tag="qT_f8")
        for hb in range(HB):
            pq = psO.tile([128, SLAB], F32, tag="po", name=f"pq{hb}")
            for ic in range(IC):
                for hpb in range(HB):
                    nc.tensor.matmul(
                        pq[:, ic * 512:(ic + 1) * 512],
                        wq_sb[hpb][:, hb * 128:(hb + 1) * 128],
                        qf[hpb][:, ic * 512:(ic + 1) * 512],
                        start=(hpb == 0), stop=(hpb == HB - 1),
                    )
            if hb == 0:
                nc.scalar.add(qT_f8[:, hb, :], pq[:, :], bq_t[hb][:])
            else:
                nc.vector.tensor_scalar_add(
                    qT_f8[:, hb, :], pq[:, :], bq_t[hb][:]
                )

        kT_f8 = kqp.tile([128, 2, N], E4, tag="kT_f8")
        vp_tiles = [None] * NP
        at_tiles = [None] * NP

        def emit_kv_dma(c):
            kc = kcp.tile([128, 2, CW], E4, tag="kc", name="kc")
            nc.sync.dma_start(kc[:], keyT8[:, :, c * CW:(c + 1) * CW])
            vc = []
            for hpb in range(HB):
                v = vcp.tile([128, CW], BF16, tag=f"vc{hpb}", name=f"vc{hpb}")
                nc.sync.dma_start(
                    v[:], valueT[hpb * 128:(hpb + 1) * 128, c * CW:(c + 1) * CW]
                )
                vc.append(v)
            return kc, vc

        def emit_kT_quarter(c, kc, q, on_act=False, pool=None):
            """Project (h-block, s-half) q of key chunk c: one psum grab."""
            hb, s = q // 2, q % 2
            if pool is None:
                pk = psP.tile([128, 512], F32, tag="pp", name="pk")
            else:
                pk = pool.tile([128, 1024], F32, tag="ps", name="pkL")[:, 0:512]
            nc.tensor.matmul(
                pk[:, :],
                wk8[:, :, hb * 128:(hb + 1) * 128],
                kc[:, :, s * 512:(s + 1) * 512],
                start=True, stop=True, perf_mode=DR,
            )
            lo = c * CW + s * 512
            if on_act:
                nc.scalar.add(kT_f8[:, hb, lo:lo + 512], pk[:, :], bk_t[hb][:])
            else:
                nc.vector.tensor_scalar_add(
                    kT_f8[:, hb, lo:lo + 512], pk[:, :], bk_t[hb][:]
                )

        def emit_v_pair(c, vc, t, pool=None):
            """Project j-pair t of value chunk c: one psum grab."""
            if pool is None:
                pv = psP.tile([128, 512], F32, tag="pp", name="pv")
            else:
                pv = pool.tile([128, 1024], F32, tag="ps", name="pvL")[:, 0:512]
            for o in range(2):
                jl = 2 * t + o
                for hpb in range(HB):
                    nc.tensor.matmul(
                        pv[:, o * 256:(o + 1) * 256],
                        vc[hpb][:, jl * 128:(jl + 1) * 128],
                        wv_sb[hpb][:, :],
                        start=(hpb == 0), stop=(hpb == HB - 1),
                    )
            p = c * PPC + t
            vp = vpp.tile([128, 2, H], E4, tag="vp", name="vp")
            nc.vector.tensor_add(vp[:, :, :], pv[:, :], bv2_t[:, :])
            vp_tiles[p] = vp

        def emit_pair_logits(p):
            at = atp.tile([128, 2, SLAB], E4, tag="at", name="at")
            for o in range(2):
                j = 2 * p + o
                pl = psL.tile([128, 1024], F32, tag="ps", name="pl")
                for ic in range(IC):
                    nc.tensor.matmul(
                        pl[:, ic * 512:(ic + 1) * 512],
                        kT_f8[:, :, j * 128:(j + 1) * 128],
                        qT_f8[:, :, ic * 512:(ic + 1) * 512],
                        start=True, stop=True, perf_mode=DR,
                    )
                nc.scalar.activation(at[:, o, :], pl[:, :], AF.Sigmoid)
            at_tiles[p] = at

        def emit_out_acc_hb0(p):
            at, vp = at_tiles[p], vp_tiles[p]
            for ic in range(IC):
                nc.tensor.matmul(
                    po[:, ic * 512:(ic + 1) * 512],
                    vp[:, :, 0:128],
                    at[:, :, ic * 512:(ic + 1) * 512],
                    start=(p == 0), stop=(p == NP - 1), perf_mode=DR,
                )

        # ---- prologue: chunk 0 projections (DMAs already in flight);
        # casts split across ACT and DVE to shorten the serial chain ----
        for q in range(4):
            emit_kT_quarter(0, kc0, q, on_act=(q % 2 == 0))
        for t in range(PPC):
            emit_v_pair(0, vc0, t)

        # the inline h-block-0 accumulator reuses the qT psum slot
        po = psO.tile([128, SLAB], F32, tag="po", name="po")

        # ---- main loop: psL carries only logits (sigmoid-paced ring);
        # one kT-quarter and one v-pair of chunk c+1 slot in per pair ----
        for c in range(NCH):
            nxt = None
            if c + 1 < NCH:
                nxt = emit_kv_dma(c + 1)
            for t in range(PPC):
                p = c * PPC + t
                emit_pair_logits(p)
                if p >= 1:
                    emit_out_acc_hb0(p - 1)
                if nxt is not None:
                    kc, vc = nxt
                    emit_kT_quarter(c + 1, kc, t)
                    emit_v_pair(c + 1, vc, t)
        emit_out_acc_hb0(NP - 1)

        # ---- phase B: h-block-1 accumulation in the freed psP slots,
        # re-reading the retained attnT/vp tiles ----
        po1 = [psP.tile([128, 512], F32, tag="pp", name=f"po1_{ic}")
               for ic in range(IC)]
        for p in range(NP):
            at, vp = at_tiles[p], vp_tiles[p]
            for ic in range(IC):
                nc.tensor.matmul(
                    po1[ic][:, :],
                    vp[:, :, 128:256],
                    at[:, :, ic * 512:(ic + 1) * 512],
                    start=(p == 0), stop=(p == NP - 1), perf_mode=DR,
                )

        # ---- drain: half-copies split DVE/ACT, 4 DMAs on 2 queues ----
        ot = [outp.tile([128, SLAB], F32, tag=f"ot{hb}", name=f"ot{hb}")
              for hb in range(HB)]
        nc.vector.tensor_copy(ot[0][:, 0:512], po[:, 0:512])
        nc.scalar.copy(ot[0][:, 512:1024], po[:, 512:1024])
        nc.vector.tensor_copy(ot[1][:, 0:512], po1[0][:, :])
        nc.scalar.copy(ot[1][:, 512:1024], po1[1][:, :])
        for hb in range(HB):
            dma = nc.sync if hb == 0 else nc.scalar
            for s in range(2):
                dma.dma_start(
                    outd[hb * 128:(hb + 1) * 128, s * 512:(s + 1) * 512],
                    ot[hb][:, s * 512:(s + 1) * 512],
                )

    nc.finalize()
    return nc


import numpy as np
import ml_dtypes
from concourse.bass_utils import run_bass_kernel_spmd

BF = ml_dtypes.bfloat16
N_CORES = 8
N_FULL = 8192
H_FULL = 256
SLAB_FULL = N_FULL // N_CORES

_NC = None


def _get_nc():
    global _NC
    if _NC is None:
        _NC = _build_attn_kernel(SLAB=SLAB_FULL, N=N_FULL, H=H_FULL)
    return _NC


def _in_maps(inputs):
    import concourse.mybir as mybir
    E4NP = mybir.dt.np(mybir.dt.float8e4)
    full = {k: np.asarray(v, dtype=np.float32) for k, v in inputs.items()}
    queryT = np.ascontiguousarray(full["query"].T.astype(BF))  # [H, N]
    N, H = full["key"].shape[0], full["key"].shape[1]
    # key/Wk in fp8 DoubleRow layout [128, 2, cols]: plane o holds
    # contraction rows 128o..128o+127
    keyT8 = full["key"].T.reshape(2, 128, N).transpose(1, 0, 2)
    WkT8 = full["Wk"].T.reshape(2, 128, H).transpose(1, 0, 2)
    shared = {
        "keyT8": np.ascontiguousarray(keyT8.astype(E4NP)),
        "valueT": np.ascontiguousarray(full["value"].T.astype(BF)),
        "WqT": np.ascontiguousarray(full["Wq"].T.astype(BF)),
        "WkT8": np.ascontiguousarray(WkT8.astype(E4NP)),
        "WvT": np.ascontiguousarray(full["Wv"].T.astype(BF)),
        "bq": np.ascontiguousarray(full["bq"]),
        "bk": np.ascontiguousarray(full["bk"]),
        "bv2": np.ascontiguousarray(np.tile(full["bv"][None, :], (128, 2))),
    }
    maps = []
    for c in range(N_CORES):
        m = dict(shared)
        m["queryT"] = np.ascontiguousarray(
            queryT[:, c * SLAB_FULL:(c + 1) * SLAB_FULL]
        )
        maps.append(m)
    return maps


def kernel(**inputs) -> np.ndarray:
    nc = _get_nc()
    res = run_bass_kernel_spmd(nc, _in_maps(inputs), list(range(N_CORES)))
    return np.ascontiguousarray(np.concatenate(
        [np.asarray(res.results[c]["outT"]).T for c in range(N_CORES)],
        axis=0,
    )).astype(np.float32)
